# revision 11
# baseline (speedup 1.0000x reference)
"""Trainium2 Bass kernel for CustomFlaxViTSelfAttention (B=64, S=577, D=768, H=12).

Strategy: data-parallel over batch across 8 NeuronCores (8 batches/core).
Per core, per batch (all matmuls bf16 on the PE, fp32 PSUM accumulate):
  - X^T tiles loaded via DMA xbar transpose (host pre-pads S 577->640 and
    casts hidden_states to bf16).
  - qT/kT computed transposed ([n, s], head-paired [128, S] tiles); V (and
    K for uniform heads) computed natural ([t, n]).
  - Heads are host-permuted to [uniform..., relu...]; output unpermutes at
    evict time (per-head dest column block), so no host-side gather.
  - relu branch: scoresT[t, s] per head -> relu -> bf16 SBUF; PV matmul with
    a literal ones-column appended to V gives both O and the L1 rowsum in
    one PSUM tile; an eps row memset into the relu'd scores makes the
    rowsum come out as (sum + 1e-5) exactly; normalize with per-partition
    reciprocal scale during PSUM->SBUF evict.
  - uniform branch: O_u = (q/8) @ (K^T V / S) -- rank-64 shortcut, no SxS.
"""

import sys

sys.path.insert(0, "/opt/trn_rl_repo")

import numpy as np
import ml_dtypes

import concourse.bass as bass  # noqa: F401  (import keeps bass registered)
import concourse.mybir as mybir
import concourse.tile as tile
from concourse import bacc
from concourse.bass_utils import run_bass_kernel_spmd

B, S, D, H, HD = 64, 577, 768, 12, 64
S_PAD = 640                  # dma_start_transpose needs free dim % 128 == 0
N_CORES = 8
B_PC = B // N_CORES
KT = D // 128                # 6 contraction tiles
NT = (S + 127) // 128        # 5 token tiles (128,128,128,128,65)
EPS = 1e-5
BF16 = mybir.dt.bfloat16
F32 = mybir.dt.float32
Copy = mybir.ActivationFunctionType.Copy
Relu = mybir.ActivationFunctionType.Relu

S_CHUNKS = [(i * 128, min(128, S - i * 128)) for i in range(NT)]     # M-dim tiles
N_CHUNKS = [(0, 512), (512, S - 512)]                                # PSUM-bank N tiles


class _Alt:
    """Round-robin DVE/ACT so elementwise work splits across both engines."""

    def __init__(self, nc):
        self.nc, self.i = nc, 0

    def copy(self, out, in_):
        self.i += 1
        if self.i % 2:
            self.nc.vector.tensor_copy(out, in_)
        else:
            self.nc.scalar.activation(out, in_, Copy)

    def relu(self, out, in_):
        self.i += 1
        if self.i % 2:
            self.nc.vector.tensor_scalar_max(out, in_, 0.0)
        else:
            self.nc.scalar.activation(out, in_, Relu)

    def scale(self, out, in_, scale_ap):
        self.i += 1
        if self.i % 2:
            self.nc.vector.tensor_scalar_mul(out, in_, scale_ap)
        else:
            self.nc.scalar.activation(out, in_, Copy, scale=scale_ap)

    def scale_const(self, out, in_, c):
        self.i += 1
        if self.i % 2:
            self.nc.vector.tensor_scalar_mul(out, in_, float(c))
        else:
            self.nc.scalar.activation(out, in_, Copy, scale=float(c))


def _groups(n, cap):
    """Split range(n) into chunks of size <= cap."""
    out, i = [], 0
    while i < n:
        out.append(list(range(i, min(i + cap, n))))
        i += cap
    return out


def build(mask, b_pc=B_PC, stage=5):
    """Build the per-core SPMD program. mask: tuple of 12 bools (True=relu).

    stage (debug bisection): 1=projections, 2=+G, 3=+scores, 4=+relu-out, 5=full.
    """
    uniform = [h for h in range(H) if not mask[h]]
    relu_heads = [h for h in range(H) if mask[h]]
    perm = uniform + relu_heads          # processed order -> original head
    nu, nr = len(uniform), len(relu_heads)

    nc = bacc.Bacc("TRN2", target_bir_lowering=False, debug=False,
                   num_devices=N_CORES)
    hs = nc.dram_tensor("hs", [b_pc, S_PAD, D], BF16, kind="ExternalInput")
    wq_d = nc.dram_tensor("wq", [D, D], BF16, kind="ExternalInput")
    wk_d = nc.dram_tensor("wk", [D, D], BF16, kind="ExternalInput")
    wv_d = nc.dram_tensor("wv", [D, D], BF16, kind="ExternalInput")
    out_d = nc.dram_tensor("out", [b_pc, S, D], F32, kind="ExternalOutput")

    # kT M-tiles: 128-col blocks (aligned with qT pairing parity) that touch
    # the relu block [64*nu, 768).
    kt_mtiles = [m for m in range(KT) if 128 * m + 128 > 64 * nu] if nr else []

    ugroups = _groups(nu, 6)   # uniform-head groups (PSUM: 64*6*4B <= 1 bank)
    rgroups = _groups(nr, 6)   # relu-head groups   (PSUM: 65*6*4B <= 1 bank)

    with (
        tile.TileContext(nc) as tc,
        tc.tile_pool(name="w", bufs=1) as pw,
        tc.tile_pool(name="x", bufs=2) as px,
        tc.tile_pool(name="qkv", bufs=2) as pqkv,
        tc.tile_pool(name="rl", bufs=1) as prl,
        tc.tile_pool(name="o", bufs=2) as po,
        tc.tile_pool(name="psA", bufs=2, space="PSUM") as psA,
        tc.tile_pool(name="psO", bufs=2, space="PSUM") as psO,
        tc.tile_pool(name="psG", bufs=2, space="PSUM") as psG,
    ):
        alt = _Alt(nc)

        # ---- weights, loaded once: [128 k-part, KT k-tile, 768 out-col] ----
        wq = pw.tile([128, KT, D], BF16, tag="wq")
        wk = pw.tile([128, KT, D], BF16, tag="wk")
        wv = pw.tile([128, KT, D], BF16, tag="wv")
        for wt, wd in ((wq, wq_d), (wk, wk_d), (wv, wv_d)):
            nc.sync.dma_start(out=wt[:], in_=wd[:].rearrange("(kt k) n -> k kt n", k=128))

        for b in range(b_pc):
            # ---- X^T via xbar transpose: 6 tiles [128 k, 640 s] bf16 ----
            xts = []
            for j in range(KT):
                xt = px.tile([128, S_PAD], BF16, tag=f"xt{j}")
                nc.sync.dma_start_transpose(xt[:], hs[b, :, 128 * j:128 * (j + 1)])
                xts.append(xt)

            # ---- qT: 6 head-paired tiles [128, S] bf16 (head h -> tile h//2, half h%2)
            qts = []
            for m in range(KT):
                ps = psA.tile([128, S], F32, tag="big")
                for k in range(KT):
                    for s0, w in N_CHUNKS:
                        nc.tensor.matmul(ps[:, s0:s0 + w],
                                         wq[:, k, 128 * m:128 * (m + 1)],
                                         xts[k][:, s0:s0 + w],
                                         start=(k == 0), stop=(k == KT - 1))
                qt = pqkv.tile([128, S], BF16, tag=f"qt{m}")
                alt.copy(qt[0:64, :], ps[0:64, :])
                alt.copy(qt[64:128, :], ps[64:128, :])
                qts.append(qt)

            # ---- kT for relu heads: same pairing as qT ----
            kts = {}   # m -> [128, S] bf16 tile
            for m in kt_mtiles:
                ps = psA.tile([128, S], F32, tag="big")
                for k in range(KT):
                    for s0, w in N_CHUNKS:
                        nc.tensor.matmul(ps[:, s0:s0 + w],
                                         wk[:, k, 128 * m:128 * (m + 1)],
                                         xts[k][:, s0:s0 + w],
                                         start=(k == 0), stop=(k == KT - 1))
                kt = pqkv.tile([128, S], BF16, tag=f"kt{m}")
                alt.copy(kt[0:64, :], ps[0:64, :])
                alt.copy(kt[64:128, :], ps[64:128, :])
                kts[m] = kt

            # ---- V natural [t, n] + literal ones-columns: [128, 12, 65] bf16
            vts = []
            for t in range(NT):
                ps = psA.tile([128, D], F32, tag="big")
                for k in range(KT):
                    for c0, cw in ((0, 512), (512, 256)):
                        nc.tensor.matmul(ps[:, c0:c0 + cw],
                                         xts[k][:, 128 * t:128 * (t + 1)],
                                         wv[:, k, c0:c0 + cw],
                                         start=(k == 0), stop=(k == KT - 1))
                vt = px.tile([128, H, HD + 1], BF16, tag=f"vt{t}")
                alt.copy(vt[:, :, 0:HD], ps[:].rearrange("p (h d) -> p h d", d=HD))
                nc.gpsimd.memset(vt[:, :, HD:HD + 1], 1.0)
                vts.append(vt)

            # ---- K natural for uniform heads (per group) + G = K^T V / S ----
            g_tiles = {}   # u -> (tile, half) ; paired like qT parity
            for grp in (ugroups if stage >= 2 else []):
                gw = 64 * len(grp)
                knats = []
                for t in range(NT):
                    ps = psO.tile([128, 512], F32, tag="O")
                    for k in range(KT):
                        nc.tensor.matmul(ps[:, 0:gw],
                                         xts[k][:, 128 * t:128 * (t + 1)],
                                         wk[:, k, 64 * grp[0]:64 * grp[0] + gw],
                                         start=(k == 0), stop=(k == KT - 1))
                    kn = pqkv.tile([128, 384], BF16, tag=f"kn{t}")
                    alt.copy(kn[:, 0:gw], ps[:, 0:gw])
                    knats.append(kn)
                for ui, u in enumerate(grp):
                    half = u % 2
                    gp = psG.tile([128, HD], F32, tag="G")
                    for t in range(NT):
                        nc.tensor.matmul(gp[64 * half:64 * half + 64, :],
                                         knats[t][:, 64 * ui:64 * ui + 64],
                                         vts[t][:, u, 0:HD],
                                         start=(t == 0), stop=(t == NT - 1))
                    gt = po.tile([128, HD], BF16, tag=f"g{u // 2}")
                    alt.scale_const(gt[64 * half:64 * half + 64, :],
                                    gp[64 * half:64 * half + 64, :], 1.0 / S)
                    g_tiles[u] = gt

            # ---- relu heads: scoresT -> relu(bf16) -> PV with ones-col ----
            rls = {}   # (r, t) -> relu'd scoresT tile [t-part, s-free]
            for r in (range(nr) if stage >= 3 else []):
                h = nu + r
                m, half = h // 2, h % 2
                for t in range(NT):
                    t0, tw = 128 * t, min(128, S - 128 * t)
                    ps = psA.tile([128, S], F32, tag="big")
                    for s0, w in N_CHUNKS:
                        nc.tensor.matmul(ps[0:tw, s0:s0 + w],
                                         kts[m][64 * half:64 * half + 64, t0:t0 + tw],
                                         qts[m][64 * half:64 * half + 64, s0:s0 + w],
                                         start=True, stop=True)
                    rl = prl.tile([tw, S], BF16, tag=f"rl{r}_{t}")
                    alt.relu(rl[0:tw, :], ps[0:tw, :])
                    rls[(r, t)] = rl

            # ---- outputs per s-tile ----
            for si, (s0, sw) in enumerate(S_CHUNKS):
                stg = po.tile([128, D], F32, tag=f"st{si}")
                if stage < 5:
                    nc.vector.memset(stg[:], 0.0)

                for grp in (rgroups if stage >= 4 else []):
                    op = psO.tile([128, 512], F32, tag="O")
                    for ri, r in enumerate(grp):
                        h = nu + r
                        for t in range(NT):
                            kk = 128 if t < NT - 1 else S - 512
                            nc.tensor.matmul(op[0:sw, 65 * ri:65 * ri + 65],
                                             rls[(r, t)][0:kk, s0:s0 + sw],
                                             vts[t][0:kk, h, :],
                                             start=(t == 0), stop=(t == NT - 1))
                    rec = po.tile([128, 8], F32, tag="rec")
                    ng = len(grp)
                    op3 = op[0:sw, 0:65 * ng].rearrange("p (r c) -> p r c", c=65)
                    nc.vector.tensor_scalar_add(rec[0:sw, 0:ng], op3[:, :, HD], EPS)
                    nc.vector.reciprocal(rec[0:sw, 0:ng], rec[0:sw, 0:ng])
                    for ri, r in enumerate(grp):
                        g0 = 64 * perm[nu + r]
                        alt.scale(stg[0:sw, g0:g0 + HD],
                                  op[0:sw, 65 * ri:65 * ri + HD],
                                  rec[0:sw, ri:ri + 1])

                # NB: matmuls with different PE row groups (parity halves) run
                # concurrently -> their drains must target different PSUM
                # banks, so split the uniform heads by parity.
                for grp in (ugroups if stage >= 5 else []):
                    for par in (0, 1):
                        pgrp = [u for u in grp if u % 2 == par]
                        if not pgrp:
                            continue
                        op = psO.tile([128, 512], F32, tag="O")
                        for ui, u in enumerate(pgrp):
                            nc.tensor.matmul(op[0:sw, 64 * ui:64 * ui + 64],
                                             qts[u // 2][64 * par:64 * par + 64, s0:s0 + sw],
                                             g_tiles[u][64 * par:64 * par + 64, :],
                                             start=True, stop=True)
                        for ui, u in enumerate(pgrp):
                            g0 = 64 * perm[u]
                            alt.copy(stg[0:sw, g0:g0 + HD],
                                     op[0:sw, 64 * ui:64 * ui + 64])

                nc.gpsimd.dma_start(out=out_d[b, s0:s0 + sw, :], in_=stg[0:sw, :])

    nc.compile()
    return nc


_CACHE = {}


def _get_nc(mask, b_pc=B_PC):
    key = (mask, b_pc)
    if key not in _CACHE:
        _CACHE[key] = build(mask, b_pc)
    return _CACHE[key]


def prep_inputs(hidden_states, Wq, Wk, Wv, mask):
    """Host-side prep: head permutation, 1/sqrt(hd) fold, bf16 cast, S pad."""
    uniform = [h for h in range(H) if not mask[h]]
    relu_heads = [h for h in range(H) if mask[h]]
    perm = uniform + relu_heads
    cols = np.concatenate([np.arange(64 * h, 64 * h + 64) for h in perm])
    wq_p = np.ascontiguousarray(
        (np.asarray(Wq, np.float32)[:, cols] * 0.125).astype(ml_dtypes.bfloat16))
    wk_p = np.ascontiguousarray(
        np.asarray(Wk, np.float32)[:, cols].astype(ml_dtypes.bfloat16))
    wv_p = np.ascontiguousarray(
        np.asarray(Wv, np.float32)[:, cols].astype(ml_dtypes.bfloat16))
    hsf = np.asarray(hidden_states, np.float32)
    hs_p = np.zeros((hsf.shape[0], S_PAD, D), ml_dtypes.bfloat16)
    hs_p[:, :S] = hsf.astype(ml_dtypes.bfloat16)
    return hs_p, wq_p, wk_p, wv_p


def make_in_maps(hidden_states, Wq, Wk, Wv, mask, b_pc=B_PC):
    hs_p, wq_p, wk_p, wv_p = prep_inputs(hidden_states, Wq, Wk, Wv, mask)
    n_shards = hs_p.shape[0] // b_pc
    return [
        {"hs": hs_p[c * b_pc:(c + 1) * b_pc], "wq": wq_p, "wk": wk_p, "wv": wv_p}
        for c in range(n_shards)
    ]


def kernel(hidden_states, Wq, bq, Wk, bk, Wv, bv, head_mask, layer_count=None, **_):
    for bias in (bq, bk, bv):
        assert not np.any(np.asarray(bias)), "nonzero qkv biases unsupported"
    mask = tuple(bool(x) for x in np.asarray(head_mask).reshape(-1))
    assert len(mask) == H

    nc = _get_nc(mask)
    in_maps = make_in_maps(hidden_states, Wq, Wk, Wv, mask)
    res = run_bass_kernel_spmd(nc, in_maps, list(range(N_CORES)))
    out = np.concatenate([res.results[c]["out"] for c in range(N_CORES)], axis=0)
    return np.ascontiguousarray(out.astype(np.float32))


# revision 22
# speedup vs baseline: 6.8122x; 6.8122x over previous
"""Trainium2 Bass kernel for CustomFlaxViTSelfAttention (B=64, S=577, D=768, H=12).

Strategy: data-parallel over batch across 8 NeuronCores (8 batches/core).
Per core, per batch (all matmuls bf16 on the PE, fp32 PSUM accumulate):
  - X^T tiles loaded via DMA xbar transpose (host pre-pads S 577->640 and
    casts hidden_states to bf16).
  - qT/kT computed transposed ([n, s], head-paired [128, S] tiles); V (and
    K for uniform heads) computed natural ([t, n]).
  - Heads are host-permuted to [uniform..., relu...]; output unpermutes at
    evict time (per-head dest column block), so no host-side gather.
  - relu branch: scoresT[t, s] per head -> relu -> bf16 SBUF; PV matmul with
    a literal ones-column appended to V gives both O and the L1 rowsum in
    one PSUM tile; an eps row memset into the relu'd scores makes the
    rowsum come out as (sum + 1e-5) exactly; normalize with per-partition
    reciprocal scale during PSUM->SBUF evict.
  - uniform branch: O_u = (q/8) @ (K^T V / S) -- rank-64 shortcut, no SxS.
"""

import sys

sys.path.insert(0, "/opt/trn_rl_repo")

import numpy as np
import ml_dtypes

import concourse.bass as bass  # noqa: F401  (import keeps bass registered)
import concourse.mybir as mybir
import concourse.tile as tile
from concourse import bacc
from concourse.bass_utils import run_bass_kernel_spmd

B, S, D, H, HD = 64, 577, 768, 12, 64
S_PAD = 640                  # dma_start_transpose needs free dim % 128 == 0
N_CORES = 8
B_PC = B // N_CORES
KT = D // 128                # 6 contraction tiles
NT = (S + 127) // 128        # 5 token tiles (128,128,128,128,65)
EPS = 1e-5
BF16 = mybir.dt.bfloat16
F32 = mybir.dt.float32
Copy = mybir.ActivationFunctionType.Copy
Relu = mybir.ActivationFunctionType.Relu

S_CHUNKS = [(i * 128, min(128, S - i * 128)) for i in range(NT)]     # M-dim tiles
N_CHUNKS = [(0, 512), (512, S - 512)]                                # PSUM-bank N tiles


class _Alt:
    """Round-robin DVE/ACT so elementwise work splits across both engines."""

    def __init__(self, nc):
        self.nc, self.i = nc, 0

    def copy(self, out, in_):
        self.i += 1
        if self.i % 2:
            self.nc.vector.tensor_copy(out, in_)
        else:
            self.nc.scalar.activation(out, in_, Copy)

    def relu(self, out, in_):
        self.i += 1
        if self.i % 2:
            self.nc.vector.tensor_scalar_max(out, in_, 0.0)
        else:
            self.nc.scalar.activation(out, in_, Relu)

    def scale(self, out, in_, scale_ap):
        self.i += 1
        if self.i % 2:
            self.nc.vector.tensor_scalar_mul(out, in_, scale_ap)
        else:
            self.nc.scalar.activation(out, in_, Copy, scale=scale_ap)

    def scale_const(self, out, in_, c):
        self.i += 1
        if self.i % 2:
            self.nc.vector.tensor_scalar_mul(out, in_, float(c))
        else:
            self.nc.scalar.activation(out, in_, Copy, scale=float(c))


def _groups(n, cap):
    """Split range(n) into chunks of size <= cap."""
    out, i = [], 0
    while i < n:
        out.append(list(range(i, min(i + cap, n))))
        i += cap
    return out


def build(mask, b_pc=B_PC, stage=5, repeat=1, loop_repeat=1):
    """Build the per-core SPMD program. mask: tuple of 12 bools (True=relu).

    stage (debug bisection): 1=projections, 2=+G, 3=+scores, 4=+relu-out, 5=full.
    repeat: run the whole batch loop N times (timing: slope over N cancels
    per-launch dispatch overhead).
    """
    uniform = [h for h in range(H) if not mask[h]]
    relu_heads = [h for h in range(H) if mask[h]]
    perm = uniform + relu_heads          # processed order -> original head
    nu, nr = len(uniform), len(relu_heads)

    nc = bacc.Bacc("TRN2", target_bir_lowering=False, debug=False,
                   num_devices=N_CORES)
    hs = nc.dram_tensor("hs", [b_pc, S_PAD, D], BF16, kind="ExternalInput")
    wq_d = nc.dram_tensor("wq", [D, D], BF16, kind="ExternalInput")
    wk_d = nc.dram_tensor("wk", [D, D], BF16, kind="ExternalInput")
    wv_d = nc.dram_tensor("wv", [D, D], BF16, kind="ExternalInput")
    out_d = nc.dram_tensor("out", [b_pc, S, D], F32, kind="ExternalOutput")

    # kT M-tiles: 128-col blocks (aligned with qT pairing parity) that touch
    # the relu block [64*nu, 768).
    kt_mtiles = [m for m in range(KT) if 128 * m + 128 > 64 * nu] if nr else []

    ugroups = _groups(nu, 6)   # uniform-head groups (PSUM: 64*6*4B <= 1 bank)
    rgroups = _groups(nr, 6)   # relu-head groups   (PSUM: 65*6*4B <= 1 bank)

    with (
        tile.TileContext(nc) as tc,
        tc.tile_pool(name="w", bufs=1) as pw,
        tc.tile_pool(name="x", bufs=2) as px,
        tc.tile_pool(name="qkv", bufs=2) as pqkv,
        tc.tile_pool(name="rl", bufs=1) as prl,
        tc.tile_pool(name="o", bufs=2) as po,
        tc.tile_pool(name="psA", bufs=3, space="PSUM") as psA,
        tc.tile_pool(name="psO", bufs=2, space="PSUM") as psO,
    ):
        psS = psA
        psG = psO
        alt = _Alt(nc)
        import contextlib
        loop_ctx = tc.For_i(0, loop_repeat, 1) if loop_repeat > 1 else contextlib.nullcontext()

        # ---- weights, loaded once: [128 k-part, KT k-tile, 768 out-col] ----
        wq = pw.tile([128, KT, D], BF16, tag="wq")
        wk = pw.tile([128, KT, D], BF16, tag="wk")
        wv = pw.tile([128, KT, D], BF16, tag="wv")
        for wt, wd in ((wq, wq_d), (wk, wk_d), (wv, wv_d)):
            nc.sync.dma_start(out=wt[:], in_=wd[:].rearrange("(kt k) n -> k kt n", k=128))

        with loop_ctx:
         for b in [bb for _ in range(repeat) for bb in range(b_pc)]:
            # ---- X^T via xbar transpose: 6 tiles [128 k, 640 s] bf16 ----
            xts = []
            for j in range(KT):
                xt = px.tile([128, S_PAD], BF16, tag=f"xt{j}")
                nc.sync.dma_start_transpose(xt[:], hs[b, :, 128 * j:128 * (j + 1)])
                xts.append(xt)

            # ---- qT: 6 head-paired tiles [128, S] bf16 (head h -> tile h//2, half h%2)
            qts = []
            for m in (range(KT) if stage >= 1 else []):
                ps = psA.tile([128, S], F32, tag="big")
                for k in range(KT):
                    for s0, w in N_CHUNKS:
                        nc.tensor.matmul(ps[:, s0:s0 + w],
                                         wq[:, k, 128 * m:128 * (m + 1)],
                                         xts[k][:, s0:s0 + w],
                                         start=(k == 0), stop=(k == KT - 1))
                qt = pqkv.tile([128, S], BF16, tag=f"qt{m}")
                alt.copy(qt[:, :], ps[:, :])
                qts.append(qt)

            # ---- kT for relu heads: same pairing as qT ----
            kts = {}   # m -> [128, S] bf16 tile
            for m in (kt_mtiles if stage >= 1 else []):
                ps = psA.tile([128, S], F32, tag="big")
                for k in range(KT):
                    for s0, w in N_CHUNKS:
                        nc.tensor.matmul(ps[:, s0:s0 + w],
                                         wk[:, k, 128 * m:128 * (m + 1)],
                                         xts[k][:, s0:s0 + w],
                                         start=(k == 0), stop=(k == KT - 1))
                kt = pqkv.tile([128, S], BF16, tag=f"kt{m}")
                alt.copy(kt[:, :], ps[:, :])
                kts[m] = kt

            # ---- V natural [t, n] + literal ones-columns: [128, 12, 65] bf16
            vts = []
            for t in (range(NT) if stage >= 1 else []):
                ps = psA.tile([128, D], F32, tag="big")
                for k in range(KT):
                    for c0, cw in ((0, 512), (512, 256)):
                        nc.tensor.matmul(ps[:, c0:c0 + cw],
                                         xts[k][:, 128 * t:128 * (t + 1)],
                                         wv[:, k, c0:c0 + cw],
                                         start=(k == 0), stop=(k == KT - 1))
                vt = px.tile([128, H, HD + 2], BF16, tag=f"vt{t}")
                alt.copy(vt[:, :, 0:HD], ps[:].rearrange("p (h d) -> p h d", d=HD))
                nc.gpsimd.memset(vt[:, :, HD:HD + 1], 1.0)
                vts.append(vt)

            # ---- K natural for uniform heads (per group) + G = K^T V / S ----
            g_tiles = {}   # u -> (tile, half) ; paired like qT parity
            for grp in (ugroups if stage >= 2 else []):
                gw = 64 * len(grp)
                knats = []
                for t in range(NT):
                    ps = psO.tile([128, 512], F32, tag="O")
                    for k in range(KT):
                        nc.tensor.matmul(ps[:, 0:gw],
                                         xts[k][:, 128 * t:128 * (t + 1)],
                                         wk[:, k, 64 * grp[0]:64 * grp[0] + gw],
                                         start=(k == 0), stop=(k == KT - 1))
                    kn = pqkv.tile([128, 384], BF16, tag=f"kn{t}")
                    alt.copy(kn[:, 0:gw], ps[:, 0:gw])
                    knats.append(kn)
                for ui, u in enumerate(grp):
                    half = u % 2
                    gp = psG.tile([128, 512], F32, tag="O")
                    for t in range(NT):
                        nc.tensor.matmul(gp[64 * half:64 * half + 64, 0:HD],
                                         knats[t][:, 64 * ui:64 * ui + 64],
                                         vts[t][:, u, 0:HD],
                                         start=(t == 0), stop=(t == NT - 1))
                    gt = po.tile([128, HD], BF16, tag=f"g{u // 2}")
                    alt.scale_const(gt[64 * half:64 * half + 64, :],
                                    gp[64 * half:64 * half + 64, 0:HD], 1.0 / S)
                    g_tiles[u] = gt

            # ---- relu heads: scoresT -> relu(bf16) -> PV with ones-col ----
            # Dedicated 2-bank scores PSUM pool (psS): scores go evict-bound,
            # but next-batch projections keep the PE busy via psA. Each evict
            # splits big/small chunks across DVE and ACT in parallel.
            rls = {}   # (r, t) -> relu'd scoresT tile [t-part, s-free]
            ei = 0
            for r in (range(nr) if stage >= 3 else []):
                h = nu + r
                m, half = h // 2, h % 2
                for t in range(NT):
                    t0, tw = 128 * t, min(128, S - 128 * t)
                    ps = psS.tile([128, S], F32, tag="big")
                    for s0, w in N_CHUNKS:
                        nc.tensor.matmul(ps[0:tw, s0:s0 + w],
                                         kts[m][64 * half:64 * half + 64, t0:t0 + tw],
                                         qts[m][64 * half:64 * half + 64, s0:s0 + w],
                                         start=True, stop=True)
                    rl = prl.tile([tw, S], BF16, tag=f"rl{r}_{t}")
                    alt.relu(rl[0:tw, :], ps[0:tw, :])
                    rls[(r, t)] = rl

            # ---- outputs per s-tile ----
            for si, (s0, sw) in enumerate(S_CHUNKS):
                stg = po.tile([128, D], F32, tag=f"st{si}")
                if stage < 5:
                    nc.vector.memset(stg[:], 0.0)

                for grp in (rgroups if stage >= 4 else []):
                    op = psO.tile([128, 512], F32, tag="O")
                    for ri, r in enumerate(grp):
                        h = nu + r
                        for t in range(NT):
                            kk = 128 if t < NT - 1 else S - 512
                            nc.tensor.matmul(op[0:sw, 65 * ri:65 * ri + 65],
                                             rls[(r, t)][0:kk, s0:s0 + sw],
                                             vts[t][0:kk, h, 0:HD + 1],
                                             start=(t == 0), stop=(t == NT - 1))
                    rec = po.tile([128, 8], F32, tag="rec")
                    ng = len(grp)
                    op3 = op[0:sw, 0:65 * ng].rearrange("p (r c) -> p r c", c=65)
                    nc.vector.tensor_scalar_add(rec[0:sw, 0:ng], op3[:, :, HD], EPS)
                    nc.vector.reciprocal(rec[0:sw, 0:ng], rec[0:sw, 0:ng])
                    for ri, r in enumerate(grp):
                        g0 = 64 * perm[nu + r]
                        alt.scale(stg[0:sw, g0:g0 + HD],
                                  op[0:sw, 65 * ri:65 * ri + HD],
                                  rec[0:sw, ri:ri + 1])

                # NB: matmuls with different PE row groups (parity halves) run
                # concurrently -> their drains must target different PSUM
                # banks, so split the uniform heads by parity.
                for grp in (ugroups if stage >= 5 else []):
                    for par in (0, 1):
                        pgrp = [u for u in grp if u % 2 == par]
                        if not pgrp:
                            continue
                        op = psO.tile([128, 512], F32, tag="O")
                        for ui, u in enumerate(pgrp):
                            nc.tensor.matmul(op[0:sw, 64 * ui:64 * ui + 64],
                                             qts[u // 2][64 * par:64 * par + 64, s0:s0 + sw],
                                             g_tiles[u][64 * par:64 * par + 64, :],
                                             start=True, stop=True)
                        for ui, u in enumerate(pgrp):
                            g0 = 64 * perm[u]
                            alt.copy(stg[0:sw, g0:g0 + HD],
                                     op[0:sw, 64 * ui:64 * ui + 64])

                nc.gpsimd.dma_start(out=out_d[b, s0:s0 + sw, :], in_=stg[0:sw, :])

    nc.compile()
    return nc


_CACHE = {}


def _get_nc(mask, b_pc=B_PC):
    key = (mask, b_pc)
    if key not in _CACHE:
        _CACHE[key] = build(mask, b_pc)
    return _CACHE[key]


def prep_inputs(hidden_states, Wq, Wk, Wv, mask):
    """Host-side prep: head permutation, 1/sqrt(hd) fold, bf16 cast, S pad."""
    uniform = [h for h in range(H) if not mask[h]]
    relu_heads = [h for h in range(H) if mask[h]]
    perm = uniform + relu_heads
    cols = np.concatenate([np.arange(64 * h, 64 * h + 64) for h in perm])
    wq_p = np.ascontiguousarray(
        (np.asarray(Wq, np.float32)[:, cols] * 0.125).astype(ml_dtypes.bfloat16))
    wk_p = np.ascontiguousarray(
        np.asarray(Wk, np.float32)[:, cols].astype(ml_dtypes.bfloat16))
    wv_p = np.ascontiguousarray(
        np.asarray(Wv, np.float32)[:, cols].astype(ml_dtypes.bfloat16))
    hsf = np.asarray(hidden_states, np.float32)
    hs_p = np.zeros((hsf.shape[0], S_PAD, D), ml_dtypes.bfloat16)
    hs_p[:, :S] = hsf.astype(ml_dtypes.bfloat16)
    return hs_p, wq_p, wk_p, wv_p


def make_in_maps(hidden_states, Wq, Wk, Wv, mask, b_pc=B_PC):
    hs_p, wq_p, wk_p, wv_p = prep_inputs(hidden_states, Wq, Wk, Wv, mask)
    n_shards = hs_p.shape[0] // b_pc
    return [
        {"hs": hs_p[c * b_pc:(c + 1) * b_pc], "wq": wq_p, "wk": wk_p, "wv": wv_p}
        for c in range(n_shards)
    ]


def kernel(hidden_states, Wq, bq, Wk, bk, Wv, bv, head_mask, layer_count=None, **_):
    for bias in (bq, bk, bv):
        assert not np.any(np.asarray(bias)), "nonzero qkv biases unsupported"
    mask = tuple(bool(x) for x in np.asarray(head_mask).reshape(-1))
    assert len(mask) == H

    nc = _get_nc(mask)
    in_maps = make_in_maps(hidden_states, Wq, Wk, Wv, mask)
    res = run_bass_kernel_spmd(nc, in_maps, list(range(N_CORES)))
    out = np.concatenate([res.results[c]["out"] for c in range(N_CORES)], axis=0)
    return np.ascontiguousarray(out.astype(np.float32))
